# revision 8
# baseline (speedup 1.0000x reference)
"""HAN heterogeneous-GNN message passing on 8 TRN2 NeuronCores.

Strategy (edge-parallel, sharded by destination):
- Only `writes` (author->paper) and `cites` (paper->paper) relations feed the
  output (the `written` branch is dead code in the reference).
- Each core owns a contiguous range of 6250 destination papers and processes
  exactly the edges targeting them (host sorts edges by dst = CSR layout).
- On device, per core: project node features (h = x @ W, fused per-head
  attention projections a_s/a_d via host-precomputed combined weights) into
  DRAM gather tables; per 128-edge block, indirect-DMA gather source rows,
  compute un-normalized attention weights exp(leaky_relu(a_s+a_d)) (softmax
  max-subtraction skipped: |alpha| <= ~1 by construction), scatter-add into a
  128-dst-window PSUM accumulator via one-hot matmul (numerator and denominator
  in one matmul), then normalize + relu, and fuse the semantic-attention
  (tanh mean) partial sums and final lin_W projection per window.
- Host combines: softmax over 2 semantic scores, weighted sum of per-relation
  projected outputs + lin_b. No collectives needed.
"""
import numpy as np
import ml_dtypes

import concourse.bass as bass
import concourse.tile as tile
from concourse import mybir
from concourse.bass_utils import run_bass_kernel_spmd
from concourse.masks import make_identity

P = 128
N_PAPER, N_AUTHOR = 50000, 20000
IN_DIM, HID, HEADS, EMB = 128, 128, 8, 64
D = HID // HEADS  # 16
NEG_SLOPE = 0.2
N_CORES = 8
DST_PER_CORE = N_PAPER // N_CORES  # 6250
NW = (DST_PER_CORE + P - 1) // P   # 49 windows of 128 dst nodes
CK = 8                              # blocks per chunk

f32 = mybir.dt.float32
bf16 = mybir.dt.bfloat16
i32 = mybir.dt.int32

# paper table cols: [h(128) | a_s_cites(8) | a_d_cites(8) | a_d_writes(8)]
TP_COLS = HID + 24  # 152
# author table cols: [h(128) | a_s_writes(8)]
TA_COLS = HID + 8   # 136
GCOLS = HID + 8     # gathered row: h + a_s



# --- walrus wait-slot legalization: this container's walrus codegen supports a
# single sync-wait slot per instruction (none on Drain); hoist extras onto
# standalone NoOps on the same engine. ---
_ZERO_WAIT_OPCODES = {"Drain"}


def _split_multiwaits(nc, default_max=1):
    n_split = 0
    for f in nc.m.functions:
        for bb in f.blocks:
            newlist = []
            changed = False
            for ins in bb.instructions:
                si = ins.sync_info
                waits = list(si.on_wait) if si is not None and si.on_wait else []
                cap = 0 if ins.opcode in _ZERO_WAIT_OPCODES else default_max
                if len(waits) > cap:
                    extra = waits[: len(waits) - cap]
                    keep = waits[len(waits) - cap:]
                    for j, w in enumerate(extra):
                        nop = mybir.InstNoOp(
                            name=f"{ins.name}-w{j}",
                            engine=ins.engine,
                            sync_info=mybir.SyncInfo(on_wait=[w], on_update=[]),
                        )
                        newlist.append(nop)
                        n_split += 1
                    si.on_wait = keep
                    changed = True
                newlist.append(ins)
            if changed:
                bb.instructions = newlist
    return n_split


def _fold_att(W, att):
    # A2[i, h] = sum_d W[i, h*16+d] * att[h, d]
    return np.einsum("ihd,hd->ih", W.reshape(IN_DIM, HEADS, D), att).astype(np.float32)


def _prep_edges(src, dst, n_blocks_common=None):
    """Sort by dst, shard by dst range, window into 128-dst tiles, pad blocks.
    Returns per-core dict + common per-window block counts."""
    order = np.argsort(dst, kind="stable")
    src_s, dst_s = src[order], dst[order]
    cores = []
    for c in range(N_CORES):
        lo, hi = c * DST_PER_CORE, (c + 1) * DST_PER_CORE
        a = np.searchsorted(dst_s, lo, "left")
        b = np.searchsorted(dst_s, hi, "left")
        cores.append((src_s[a:b], dst_s[a:b] - lo))
    # per-window counts
    cnt = np.zeros((N_CORES, NW), np.int64)
    for c, (_, dl) in enumerate(cores):
        w = dl // P
        np.add.at(cnt[c], w, 1)
    nbw = np.maximum(1, (cnt.max(axis=0) + P - 1) // P).astype(np.int64)  # [NW]
    nbtot = int(nbw.sum())
    out = []
    for c, (s, dl) in enumerate(cores):
        w = dl // P
        slot = (dl % P).astype(np.float32)
        src_arr = np.zeros((nbtot * P,), np.int32)
        slot_arr = np.full((nbtot * P,), -1.0, np.float32)
        ad_arr = np.zeros((nbtot * P,), np.int32)
        base = 0
        for wi in range(NW):
            sel = w == wi
            k = int(cnt[c, wi])
            src_arr[base: base + k] = s[sel]
            slot_arr[base: base + k] = slot[sel]
            ad_arr[base: base + k] = (dl[sel] + c * DST_PER_CORE).astype(np.int32)
            base += int(nbw[wi]) * P
        # transposed block layout [P, nbtot]: column j = block j
        out.append({
            "src": src_arr.reshape(nbtot, P).T.copy(),
            "slot": slot_arr.reshape(nbtot, P).T.copy(),
            "ad": ad_arr.reshape(nbtot, P).T.copy(),
        })
    return out, nbw


def _build_program(nbw_w, nbw_c):
    nbt_w, nbt_c = int(nbw_w.sum()), int(nbw_c.sum())
    nc = bass.Bass()
    # inputs
    t_xp = nc.declare_dram_parameter("xp", [IN_DIM, N_PAPER], bf16, isOutput=False)
    t_xa = nc.declare_dram_parameter("xa", [IN_DIM, N_AUTHOR], bf16, isOutput=False)
    t_wcp = nc.declare_dram_parameter("wcp", [IN_DIM, TP_COLS], bf16, isOutput=False)
    t_wca = nc.declare_dram_parameter("wca", [IN_DIM, TA_COLS], bf16, isOutput=False)
    t_kw = nc.declare_dram_parameter("kw", [HID, HID], f32, isOutput=False)
    t_kb = nc.declare_dram_parameter("kb", [HID, 1], f32, isOutput=False)
    t_lw = nc.declare_dram_parameter("lw", [HID, EMB], f32, isOutput=False)
    t_iota = nc.declare_dram_parameter("iotam", [P, P], f32, isOutput=False)
    e_in = {}
    for rel, nbt in (("w", nbt_w), ("c", nbt_c)):
        e_in[rel] = {
            "src": nc.declare_dram_parameter(f"src_{rel}", [P, nbt], i32, isOutput=False),
            "slot": nc.declare_dram_parameter(f"slot_{rel}", [P, nbt], f32, isOutput=False),
            "ad": nc.declare_dram_parameter(f"ad_{rel}", [P, nbt], i32, isOutput=False),
        }
    # outputs
    o_yw = nc.declare_dram_parameter("yw", [EMB, NW * P], f32, isOutput=True)
    o_yc = nc.declare_dram_parameter("yc", [EMB, NW * P], f32, isOutput=True)
    o_s = nc.declare_dram_parameter("sacc", [HID, 2], f32, isOutput=True)
    # internal gather tables
    T_p = nc.dram_tensor("T_p", [N_PAPER, TP_COLS], bf16)
    T_a = nc.dram_tensor("T_a", [N_AUTHOR, TA_COLS], bf16)

    with tile.TileContext(nc) as tc:
        with tc.tile_pool(name="const", bufs=1) as const:
            ident = const.tile([P, P], f32)
            make_identity(nc, ident[:])
            iota_f = const.tile([P, P], f32)
            nc.sync.dma_start(out=iota_f[:], in_=t_iota[:])
            wcp_t = const.tile([IN_DIM, TP_COLS], bf16)
            nc.sync.dma_start(out=wcp_t[:], in_=t_wcp[:])
            wca_t = const.tile([IN_DIM, TA_COLS], bf16)
            nc.sync.dma_start(out=wca_t[:], in_=t_wca[:])
            kw_t = const.tile([HID, HID], f32)
            nc.sync.dma_start(out=kw_t[:], in_=t_kw[:])
            kb_t = const.tile([HID, 1], f32)
            nc.sync.dma_start(out=kb_t[:], in_=t_kb[:])
            lw_t = const.tile([HID, EMB], f32)
            nc.sync.dma_start(out=lw_t[:], in_=t_lw[:])
            acc_w = const.tile([HID, 1], f32)
            nc.vector.memset(acc_w[:], 0.0)
            acc_c = const.tile([HID, 1], f32)
            nc.vector.memset(acc_c[:], 0.0)

            # ---------------- Phase A: build gather tables ----------------
            with tc.tile_pool(name="pa_sb", bufs=4) as pa_sb, \
                 tc.tile_pool(name="pa_ps", bufs=4, space="PSUM") as pa_ps:
                for (x_t, T_t, wc_t, nrow, ncol) in (
                    (t_xa, T_a, wca_t, N_AUTHOR, TA_COLS),
                    (t_xp, T_p, wcp_t, N_PAPER, TP_COLS),
                ):
                    ntile = (nrow + P - 1) // P
                    for ti in range(ntile):
                        r0 = ti * P
                        rows = min(P, nrow - r0)
                        xT = pa_sb.tile([P, P], bf16, tag="xT")
                        nc.sync.dma_start(out=xT[:, :rows], in_=x_t[:, r0:r0 + rows])
                        pj = pa_ps.tile([P, TP_COLS], f32, space="PSUM", tag="pj")
                        nc.tensor.matmul(out=pj[:rows, :ncol], lhsT=xT[:, :rows], rhs=wc_t[:, :ncol], start=True, stop=True)
                        ot = pa_sb.tile([P, TP_COLS], bf16, tag="ot")
                        if ti % 2 == 0:
                            nc.vector.tensor_copy(out=ot[:rows, :ncol], in_=pj[:rows, :ncol])
                        else:
                            nc.scalar.copy(out=ot[:rows, :ncol], in_=pj[:rows, :ncol])
                        nc.sync.dma_start(out=T_t[r0:r0 + rows, :], in_=ot[:rows, :ncol])

            # ---------------- Phase B: edge processing ----------------
            with tc.tile_pool(name="meta", bufs=1) as meta, \
                 tc.tile_pool(name="eb", bufs=3) as eb, \
                 tc.tile_pool(name="wps", bufs=2, space="PSUM") as wps, \
                 tc.tile_pool(name="eps", bufs=2, space="PSUM") as eps:
                for rel, nbw, T_src, ad_off, y_out, acc_rel in (
                    ("w", nbw_w, T_a, HID + 16, o_yw, acc_w),
                    ("c", nbw_c, T_p, HID + 8, o_yc, acc_c),
                ):
                    nbt = int(nbw.sum())
                    src_m = meta.tile([P, nbt], i32, tag=f"src{rel}")
                    nc.sync.dma_start(out=src_m[:], in_=e_in[rel]["src"][:])
                    slot_m = meta.tile([P, nbt], f32, tag=f"slot{rel}")
                    nc.sync.dma_start(out=slot_m[:], in_=e_in[rel]["slot"][:])
                    ad_m = meta.tile([P, nbt], i32, tag=f"ad{rel}")
                    nc.sync.dma_start(out=ad_m[:], in_=e_in[rel]["ad"][:])

                    gb = 0  # global block index
                    for wi in range(NW):
                        nb = int(nbw[wi])
                        win = wps.tile([P, GCOLS], f32, space="PSUM", tag="win")
                        done = 0
                        while done < nb:
                            ck = min(CK, nb - done)
                            G = eb.tile([P, CK, GCOLS], bf16, tag="G")
                            AD = eb.tile([P, CK, 8], bf16, tag="AD")
                            for b in range(ck):
                                col = gb + done + b
                                nc.gpsimd.indirect_dma_start(
                                    out=G[:, b, :],
                                    out_offset=None,
                                    in_=T_src[:],
                                    in_offset=bass.IndirectOffsetOnAxis(ap=src_m[:, col:col + 1], axis=0),
                                )
                                nc.gpsimd.indirect_dma_start(
                                    out=AD[:, b, :],
                                    out_offset=None,
                                    in_=T_p[:],
                                    in_offset=bass.IndirectOffsetOnAxis(ap=ad_m[:, col:col + 1], axis=0),
                                    element_offset=ad_off,
                                )
                            E = eb.tile([P, CK, 8], bf16, tag="E")
                            nc.vector.tensor_tensor(
                                out=E[:, :ck, :], in0=G[:, :ck, HID:GCOLS], in1=AD[:, :ck, :],
                                op=mybir.AluOpType.add)
                            E2 = eb.tile([P, CK, 8], bf16, tag="E2")
                            nc.vector.tensor_scalar(out=E2[:, :ck, :], in0=E[:, :ck, :],
                                                    scalar1=NEG_SLOPE, scalar2=None, op0=mybir.AluOpType.mult)
                            nc.vector.tensor_tensor(out=E[:, :ck, :], in0=E[:, :ck, :], in1=E2[:, :ck, :],
                                                    op=mybir.AluOpType.max)
                            nc.scalar.activation(out=E[:, :ck, :], in_=E[:, :ck, :],
                                                 func=mybir.ActivationFunctionType.Exp)
                            kxn = eb.tile([P, CK, GCOLS], bf16, tag="kxn")
                            oh = eb.tile([P, CK, P], bf16, tag="oh")
                            for b in range(ck):
                                nc.vector.tensor_tensor(
                                    out=kxn[:, b, 0:HID].rearrange("p (h d) -> p h d", h=HEADS),
                                    in0=G[:, b, 0:HID].rearrange("p (h d) -> p h d", h=HEADS),
                                    in1=E[:, b, :].unsqueeze(2).to_broadcast([P, HEADS, D]),
                                    op=mybir.AluOpType.mult)
                                nc.vector.tensor_tensor(
                                    out=oh[:, b, :],
                                    in0=slot_m[:, gb + done + b: gb + done + b + 1].to_broadcast([P, P]),
                                    in1=iota_f[:],
                                    op=mybir.AluOpType.is_equal)
                            nc.scalar.copy(out=kxn[:, :ck, HID:GCOLS], in_=E[:, :ck, :])
                            for b in range(ck):
                                nc.tensor.matmul(
                                    out=win[:],
                                    lhsT=oh[:, b, :],
                                    rhs=kxn[:, b, :],
                                    start=(done + b == 0),
                                    stop=(done + b == nb - 1))
                            done += ck
                        gb += nb
                        # window epilogue
                        dn = eb.tile([P, 8], f32, tag="dn")
                        nc.vector.tensor_scalar(out=dn[:], in0=win[:, HID:GCOLS],
                                                scalar1=1e-16, scalar2=None, op0=mybir.AluOpType.add)
                        rc = eb.tile([P, 8], f32, tag="rc")
                        nc.vector.reciprocal(out=rc[:], in_=dn[:])
                        orel = eb.tile([P, HID], f32, tag="orel")
                        nc.vector.tensor_tensor(
                            out=orel[:].rearrange("p (h d) -> p h d", h=HEADS),
                            in0=win[:, 0:HID].rearrange("p (h d) -> p h d", h=HEADS),
                            in1=rc[:].unsqueeze(2).to_broadcast([P, HEADS, D]),
                            op=mybir.AluOpType.mult)
                        orr = eb.tile([P, HID], f32, tag="orr")
                        nc.vector.tensor_scalar(out=orr[:], in0=orel[:],
                                                scalar1=0.0, scalar2=None, op0=mybir.AluOpType.max)
                        trp = eps.tile([P, P], f32, space="PSUM", tag="trp")
                        nc.tensor.transpose(out=trp[:], in_=orr[:], identity=ident[:])
                        R = eb.tile([P, P], f32, tag="R")
                        nc.scalar.copy(out=R[:], in_=trp[:])
                        tn = eps.tile([P, P], f32, space="PSUM", tag="tn")
                        nc.tensor.matmul(out=tn[:], lhsT=kw_t[:], rhs=R[:], start=True, stop=True)
                        tns = eb.tile([P, P], f32, tag="tns")
                        atmp = eb.tile([P, 1], f32, tag="atmp")
                        nc.scalar.activation(out=tns[:], in_=tn[:],
                                             func=mybir.ActivationFunctionType.Tanh,
                                             bias=kb_t[:], accum_out=atmp[:])
                        nc.vector.tensor_add(out=acc_rel[:], in0=acc_rel[:], in1=atmp[:])
                        yp = eps.tile([EMB, P], f32, space="PSUM", tag="yp")
                        nc.tensor.matmul(out=yp[:], lhsT=lw_t[:], rhs=R[:], start=True, stop=True)
                        ys = eb.tile([EMB, P], f32, tag="ys")
                        nc.vector.tensor_copy(out=ys[:], in_=yp[:])
                        nc.sync.dma_start(out=y_out[:, wi * P:(wi + 1) * P], in_=ys[:])

            sout = const.tile([HID, 2], f32)
            nc.vector.tensor_copy(out=sout[:, 0:1], in_=acc_w[:])
            nc.vector.tensor_copy(out=sout[:, 1:2], in_=acc_c[:])
            nc.sync.dma_start(out=o_s[:], in_=sout[:])

    _split_multiwaits(nc)
    return nc


def kernel(**inputs):
    xp = np.ascontiguousarray(inputs["x_paper"], np.float32)
    xa = np.ascontiguousarray(inputs["x_author"], np.float32)
    Wp, bp = np.asarray(inputs["W_paper"], np.float32), np.asarray(inputs["b_paper"], np.float32)
    Wa, ba = np.asarray(inputs["W_author"], np.float32), np.asarray(inputs["b_author"], np.float32)
    kW, kb = np.asarray(inputs["k_W"], np.float32), np.asarray(inputs["k_b"], np.float32)
    q = np.asarray(inputs["q"], np.float32)
    lW, lb = np.asarray(inputs["lin_W"], np.float32), np.asarray(inputs["lin_b"], np.float32)
    assert not bp.any() and not ba.any(), "nonzero node bias not supported"

    # combined projection weights
    wcp = np.concatenate([
        Wp,
        _fold_att(Wp, np.asarray(inputs["att_src_cites"], np.float32)),
        _fold_att(Wp, np.asarray(inputs["att_dst_cites"], np.float32)),
        _fold_att(Wp, np.asarray(inputs["att_dst_writes"], np.float32)),
    ], axis=1)
    wca = np.concatenate([
        Wa,
        _fold_att(Wa, np.asarray(inputs["att_src_writes"], np.float32)),
    ], axis=1)

    ew, nbw_w = _prep_edges(np.asarray(inputs["writes_src"]), np.asarray(inputs["writes_dst"]))
    ec, nbw_c = _prep_edges(np.asarray(inputs["cites_src"]), np.asarray(inputs["cites_dst"]))

    nc = _build_program(nbw_w, nbw_c)

    iotam = np.tile(np.arange(P, dtype=np.float32), (P, 1))
    common = {
        "xp": np.ascontiguousarray(xp.T.astype(ml_dtypes.bfloat16)), "xa": np.ascontiguousarray(xa.T.astype(ml_dtypes.bfloat16)),
        "wcp": wcp.astype(ml_dtypes.bfloat16), "wca": wca.astype(ml_dtypes.bfloat16),
        "kw": kW, "kb": kb.reshape(HID, 1).copy(), "lw": lW, "iotam": iotam,
    }
    in_maps = []
    for c in range(N_CORES):
        m = dict(common)
        m.update({
            "src_w": ew[c]["src"], "slot_w": ew[c]["slot"], "ad_w": ew[c]["ad"],
            "src_c": ec[c]["src"], "slot_c": ec[c]["slot"], "ad_c": ec[c]["ad"],
        })
        in_maps.append(m)

    res = run_bass_kernel_spmd(nc, in_maps, list(range(N_CORES)))
    global LAST_RES, LAST_A, LAST_SCORES
    LAST_RES = res

    # host combine: semantic attention softmax + weighted sum + lin_b
    n_pad = NW * P - DST_PER_CORE
    S = np.zeros((HID, 2), np.float64)
    for c in range(N_CORES):
        S += res.results[c]["sacc"].astype(np.float64)
    S -= N_CORES * n_pad * np.tanh(kb)[:, None]  # remove padded-column tanh(k_b)
    t_mean = S / N_PAPER  # [HID, 2]
    scores = q @ t_mean  # [2]
    e = np.exp(scores - scores.max())
    a = e / e.sum()  # [a_writes, a_cites]
    LAST_A, LAST_SCORES = a, scores

    out = np.empty((N_PAPER, EMB), np.float32)
    for c in range(N_CORES):
        yw = res.results[c]["yw"][:, :DST_PER_CORE]
        yc = res.results[c]["yc"][:, :DST_PER_CORE]
        out[c * DST_PER_CORE:(c + 1) * DST_PER_CORE] = (a[0] * yw + a[1] * yc).T
    out += lb[None, :]
    return out


# revision 9
# speedup vs baseline: 5.9545x; 5.9545x over previous
"""HAN heterogeneous-GNN message passing on 8 TRN2 NeuronCores.

Strategy (edge-parallel, sharded by destination):
- Only `writes` (author->paper) and `cites` (paper->paper) relations feed the
  output (the `written` branch is dead code in the reference).
- Each core owns a contiguous range of 6250 destination papers and processes
  exactly the edges targeting them (host sorts edges by dst = CSR layout).
- On device, per core: project node features (h = x @ W, fused per-head
  attention projections a_s/a_d via host-precomputed combined weights) into
  DRAM gather tables; per 128-edge block, indirect-DMA gather source rows,
  compute un-normalized attention weights exp(leaky_relu(a_s+a_d)) (softmax
  max-subtraction skipped: |alpha| <= ~1 by construction), scatter-add into a
  128-dst-window PSUM accumulator via one-hot matmul (numerator and denominator
  in one matmul), then normalize + relu, and fuse the semantic-attention
  (tanh mean) partial sums and final lin_W projection per window.
- Host combines: softmax over 2 semantic scores, weighted sum of per-relation
  projected outputs + lin_b. No collectives needed.
"""
import numpy as np
import ml_dtypes

import concourse.bass as bass
import concourse.tile as tile
from concourse import mybir
from concourse.bass_utils import run_bass_kernel_spmd
from concourse.masks import make_identity

P = 128
N_PAPER, N_AUTHOR = 50000, 20000
IN_DIM, HID, HEADS, EMB = 128, 128, 8, 64
D = HID // HEADS  # 16
NEG_SLOPE = 0.2
N_CORES = 8
DST_PER_CORE = N_PAPER // N_CORES  # 6250
NW = (DST_PER_CORE + P - 1) // P   # 49 windows of 128 dst nodes
CK = 16                             # blocks per chunk

f32 = mybir.dt.float32
bf16 = mybir.dt.bfloat16
i32 = mybir.dt.int32

# paper table cols: [h(128) | a_s_cites(8) | a_d_cites(8) | a_d_writes(8)]
TP_COLS = HID + 24  # 152
# author table cols: [h(128) | a_s_writes(8)]
TA_COLS = HID + 8   # 136
GCOLS = HID + 8     # gathered row: h + a_s



# --- walrus wait-slot legalization: this container's walrus codegen supports a
# single sync-wait slot per instruction (none on Drain); hoist extras onto
# standalone NoOps on the same engine. ---
_ZERO_WAIT_OPCODES = {"Drain"}


def _split_multiwaits(nc, default_max=1):
    n_split = 0
    for f in nc.m.functions:
        for bb in f.blocks:
            newlist = []
            changed = False
            for ins in bb.instructions:
                si = ins.sync_info
                waits = list(si.on_wait) if si is not None and si.on_wait else []
                cap = 0 if ins.opcode in _ZERO_WAIT_OPCODES else default_max
                if len(waits) > cap:
                    extra = waits[: len(waits) - cap]
                    keep = waits[len(waits) - cap:]
                    for j, w in enumerate(extra):
                        nop = mybir.InstNoOp(
                            name=f"{ins.name}-w{j}",
                            engine=ins.engine,
                            sync_info=mybir.SyncInfo(on_wait=[w], on_update=[]),
                        )
                        newlist.append(nop)
                        n_split += 1
                    si.on_wait = keep
                    changed = True
                newlist.append(ins)
            if changed:
                bb.instructions = newlist
    return n_split


def _fold_att(W, att):
    # A2[i, h] = sum_d W[i, h*16+d] * att[h, d]
    return np.einsum("ihd,hd->ih", W.reshape(IN_DIM, HEADS, D), att).astype(np.float32)


def _prep_edges(src, dst, n_blocks_common=None):
    """Sort by dst, shard by dst range, window into 128-dst tiles, pad blocks.
    Returns per-core dict + common per-window block counts."""
    order = np.argsort(dst, kind="stable")
    src_s, dst_s = src[order], dst[order]
    cores = []
    for c in range(N_CORES):
        lo, hi = c * DST_PER_CORE, (c + 1) * DST_PER_CORE
        a = np.searchsorted(dst_s, lo, "left")
        b = np.searchsorted(dst_s, hi, "left")
        cores.append((src_s[a:b], dst_s[a:b] - lo))
    # per-window counts
    cnt = np.zeros((N_CORES, NW), np.int64)
    for c, (_, dl) in enumerate(cores):
        w = dl // P
        np.add.at(cnt[c], w, 1)
    nbw = np.maximum(1, (cnt.max(axis=0) + P - 1) // P).astype(np.int64)  # [NW]
    nbtot = int(nbw.sum())
    out = []
    for c, (s, dl) in enumerate(cores):
        w = dl // P
        slot = (dl % P).astype(np.float32)
        src_arr = np.zeros((nbtot * P,), np.int32)
        slot_arr = np.full((nbtot * P,), -1.0, np.float32)
        ad_arr = np.zeros((nbtot * P,), np.int32)
        base = 0
        for wi in range(NW):
            sel = w == wi
            k = int(cnt[c, wi])
            src_arr[base: base + k] = s[sel]
            slot_arr[base: base + k] = slot[sel]
            ad_arr[base: base + k] = (dl[sel] + c * DST_PER_CORE).astype(np.int32)
            base += int(nbw[wi]) * P
        # transposed block layout [P, nbtot]: column j = block j
        out.append({
            "src": src_arr.reshape(nbtot, P).T.copy(),
            "slot": slot_arr.reshape(nbtot, P).T.copy(),
            "ad": ad_arr.reshape(nbtot, P).T.copy(),
        })
    return out, nbw


def _build_program(nbw_w, nbw_c):
    nbt_w, nbt_c = int(nbw_w.sum()), int(nbw_c.sum())
    nc = bass.Bass()
    # inputs
    t_xp = nc.declare_dram_parameter("xp", [IN_DIM, N_PAPER], bf16, isOutput=False)
    t_xa = nc.declare_dram_parameter("xa", [IN_DIM, N_AUTHOR], bf16, isOutput=False)
    t_wcp = nc.declare_dram_parameter("wcp", [IN_DIM, TP_COLS], bf16, isOutput=False)
    t_wca = nc.declare_dram_parameter("wca", [IN_DIM, TA_COLS], bf16, isOutput=False)
    t_kw = nc.declare_dram_parameter("kw", [HID, HID], f32, isOutput=False)
    t_kb = nc.declare_dram_parameter("kb", [HID, 1], f32, isOutput=False)
    t_lw = nc.declare_dram_parameter("lw", [HID, EMB], f32, isOutput=False)
    t_iota = nc.declare_dram_parameter("iotam", [P, P], f32, isOutput=False)
    e_in = {}
    for rel, nbt in (("w", nbt_w), ("c", nbt_c)):
        e_in[rel] = {
            "src": nc.declare_dram_parameter(f"src_{rel}", [P, nbt], i32, isOutput=False),
            "slot": nc.declare_dram_parameter(f"slot_{rel}", [P, nbt], f32, isOutput=False),
            "ad": nc.declare_dram_parameter(f"ad_{rel}", [P, nbt], i32, isOutput=False),
        }
    # outputs
    o_yw = nc.declare_dram_parameter("yw", [EMB, NW * P], f32, isOutput=True)
    o_yc = nc.declare_dram_parameter("yc", [EMB, NW * P], f32, isOutput=True)
    o_s = nc.declare_dram_parameter("sacc", [HID, 2], f32, isOutput=True)
    # internal gather tables
    T_p = nc.dram_tensor("T_p", [N_PAPER, TP_COLS], bf16)
    T_a = nc.dram_tensor("T_a", [N_AUTHOR, TA_COLS], bf16)

    with tile.TileContext(nc) as tc:
        with tc.tile_pool(name="const", bufs=1) as const:
            ident = const.tile([P, P], f32)
            make_identity(nc, ident[:])
            iota_f = const.tile([P, P], f32)
            nc.sync.dma_start(out=iota_f[:], in_=t_iota[:])
            wcp_t = const.tile([IN_DIM, TP_COLS], bf16)
            nc.sync.dma_start(out=wcp_t[:], in_=t_wcp[:])
            wca_t = const.tile([IN_DIM, TA_COLS], bf16)
            nc.sync.dma_start(out=wca_t[:], in_=t_wca[:])
            kw_t = const.tile([HID, HID], f32)
            nc.sync.dma_start(out=kw_t[:], in_=t_kw[:])
            kb_t = const.tile([HID, 1], f32)
            nc.sync.dma_start(out=kb_t[:], in_=t_kb[:])
            lw_t = const.tile([HID, EMB], f32)
            nc.sync.dma_start(out=lw_t[:], in_=t_lw[:])
            acc_w = const.tile([HID, 1], f32)
            nc.vector.memset(acc_w[:], 0.0)
            acc_c = const.tile([HID, 1], f32)
            nc.vector.memset(acc_c[:], 0.0)

            # ---------------- Phase A: build gather tables ----------------
            with tc.tile_pool(name="pa_sb", bufs=4) as pa_sb, \
                 tc.tile_pool(name="pa_ps", bufs=4, space="PSUM") as pa_ps:
                for (x_t, T_t, wc_t, nrow, ncol) in (
                    (t_xp, T_p, wcp_t, N_PAPER, TP_COLS),
                    (t_xa, T_a, wca_t, N_AUTHOR, TA_COLS),
                ):
                    ntile = (nrow + P - 1) // P
                    for ti in range(ntile):
                        r0 = ti * P
                        rows = min(P, nrow - r0)
                        xT = pa_sb.tile([P, P], bf16, tag="xT")
                        nc.sync.dma_start(out=xT[:, :rows], in_=x_t[:, r0:r0 + rows])
                        pj = pa_ps.tile([P, TP_COLS], f32, space="PSUM", tag="pj")
                        nc.tensor.matmul(out=pj[:rows, :ncol], lhsT=xT[:, :rows], rhs=wc_t[:, :ncol], start=True, stop=True)
                        ot = pa_sb.tile([P, TP_COLS], bf16, tag="ot")
                        if ti % 2 == 0:
                            nc.vector.tensor_copy(out=ot[:rows, :ncol], in_=pj[:rows, :ncol])
                        else:
                            nc.scalar.copy(out=ot[:rows, :ncol], in_=pj[:rows, :ncol])
                        nc.sync.dma_start(out=T_t[r0:r0 + rows, :], in_=ot[:rows, :ncol])

            # ---------------- Phase B: edge processing ----------------
            with tc.tile_pool(name="meta", bufs=1) as meta, \
                 tc.tile_pool(name="eb", bufs=4) as eb, \
                 tc.tile_pool(name="wps", bufs=2, space="PSUM") as wps, \
                 tc.tile_pool(name="eps", bufs=2, space="PSUM") as eps:
                for rel, nbw, T_src, ad_off, y_out, acc_rel in (
                    ("c", nbw_c, T_p, HID + 8, o_yc, acc_c),
                    ("w", nbw_w, T_a, HID + 16, o_yw, acc_w),
                ):
                    nbt = int(nbw.sum())
                    src_m = meta.tile([P, nbt], i32, tag=f"src{rel}")
                    nc.sync.dma_start(out=src_m[:], in_=e_in[rel]["src"][:])
                    slot_m = meta.tile([P, nbt], f32, tag=f"slot{rel}")
                    nc.sync.dma_start(out=slot_m[:], in_=e_in[rel]["slot"][:])
                    ad_m = meta.tile([P, nbt], i32, tag=f"ad{rel}")
                    nc.sync.dma_start(out=ad_m[:], in_=e_in[rel]["ad"][:])

                    gb = 0  # global block index
                    for wi in range(NW):
                        nb = int(nbw[wi])
                        win = wps.tile([P, GCOLS], f32, space="PSUM", tag="win")
                        done = 0
                        while done < nb:
                            ck = min(CK, nb - done)
                            G = eb.tile([P, CK, GCOLS], bf16, tag="G")
                            AD = eb.tile([P, CK, 8], bf16, tag="AD")
                            for b in range(ck):
                                col = gb + done + b
                                nc.gpsimd.indirect_dma_start(
                                    out=G[:, b, :],
                                    out_offset=None,
                                    in_=T_src[:],
                                    in_offset=bass.IndirectOffsetOnAxis(ap=src_m[:, col:col + 1], axis=0),
                                )
                                nc.gpsimd.indirect_dma_start(
                                    out=AD[:, b, :],
                                    out_offset=None,
                                    in_=T_p[:],
                                    in_offset=bass.IndirectOffsetOnAxis(ap=ad_m[:, col:col + 1], axis=0),
                                    element_offset=ad_off,
                                )
                            E = eb.tile([P, CK, 8], bf16, tag="E")
                            nc.vector.tensor_tensor(
                                out=E[:, :ck, :], in0=G[:, :ck, HID:GCOLS], in1=AD[:, :ck, :],
                                op=mybir.AluOpType.add)
                            E2 = eb.tile([P, CK, 8], bf16, tag="E2")
                            nc.vector.tensor_scalar(out=E2[:, :ck, :], in0=E[:, :ck, :],
                                                    scalar1=NEG_SLOPE, scalar2=None, op0=mybir.AluOpType.mult)
                            nc.vector.tensor_tensor(out=E[:, :ck, :], in0=E[:, :ck, :], in1=E2[:, :ck, :],
                                                    op=mybir.AluOpType.max)
                            nc.scalar.activation(out=E[:, :ck, :], in_=E[:, :ck, :],
                                                 func=mybir.ActivationFunctionType.Exp)
                            kxn = eb.tile([P, CK, GCOLS], bf16, tag="kxn")
                            oh = eb.tile([P, CK, P], bf16, tag="oh")
                            for b in range(ck):
                                nc.vector.tensor_tensor(
                                    out=kxn[:, b, 0:HID].rearrange("p (h d) -> p h d", h=HEADS),
                                    in0=G[:, b, 0:HID].rearrange("p (h d) -> p h d", h=HEADS),
                                    in1=E[:, b, :].unsqueeze(2).to_broadcast([P, HEADS, D]),
                                    op=mybir.AluOpType.mult)
                                nc.vector.tensor_tensor(
                                    out=oh[:, b, :],
                                    in0=slot_m[:, gb + done + b: gb + done + b + 1].to_broadcast([P, P]),
                                    in1=iota_f[:],
                                    op=mybir.AluOpType.is_equal)
                            nc.scalar.copy(out=kxn[:, :ck, HID:GCOLS], in_=E[:, :ck, :])
                            for b in range(ck):
                                nc.tensor.matmul(
                                    out=win[:],
                                    lhsT=oh[:, b, :],
                                    rhs=kxn[:, b, :],
                                    start=(done + b == 0),
                                    stop=(done + b == nb - 1))
                            done += ck
                        gb += nb
                        # window epilogue
                        dn = eb.tile([P, 8], f32, tag="dn")
                        nc.vector.tensor_scalar(out=dn[:], in0=win[:, HID:GCOLS],
                                                scalar1=1e-16, scalar2=None, op0=mybir.AluOpType.add)
                        rc = eb.tile([P, 8], f32, tag="rc")
                        nc.vector.reciprocal(out=rc[:], in_=dn[:])
                        orel = eb.tile([P, HID], f32, tag="orel")
                        nc.vector.tensor_tensor(
                            out=orel[:].rearrange("p (h d) -> p h d", h=HEADS),
                            in0=win[:, 0:HID].rearrange("p (h d) -> p h d", h=HEADS),
                            in1=rc[:].unsqueeze(2).to_broadcast([P, HEADS, D]),
                            op=mybir.AluOpType.mult)
                        orr = eb.tile([P, HID], f32, tag="orr")
                        nc.vector.tensor_scalar(out=orr[:], in0=orel[:],
                                                scalar1=0.0, scalar2=None, op0=mybir.AluOpType.max)
                        trp = eps.tile([P, P], f32, space="PSUM", tag="trp")
                        nc.tensor.transpose(out=trp[:], in_=orr[:], identity=ident[:])
                        R = eb.tile([P, P], f32, tag="R")
                        nc.scalar.copy(out=R[:], in_=trp[:])
                        tn = eps.tile([P, P], f32, space="PSUM", tag="tn")
                        nc.tensor.matmul(out=tn[:], lhsT=kw_t[:], rhs=R[:], start=True, stop=True)
                        tns = eb.tile([P, P], f32, tag="tns")
                        atmp = eb.tile([P, 1], f32, tag="atmp")
                        nc.scalar.activation(out=tns[:], in_=tn[:],
                                             func=mybir.ActivationFunctionType.Tanh,
                                             bias=kb_t[:], accum_out=atmp[:])
                        nc.vector.tensor_add(out=acc_rel[:], in0=acc_rel[:], in1=atmp[:])
                        yp = eps.tile([EMB, P], f32, space="PSUM", tag="yp")
                        nc.tensor.matmul(out=yp[:], lhsT=lw_t[:], rhs=R[:], start=True, stop=True)
                        ys = eb.tile([EMB, P], f32, tag="ys")
                        nc.vector.tensor_copy(out=ys[:], in_=yp[:])
                        nc.sync.dma_start(out=y_out[:, wi * P:(wi + 1) * P], in_=ys[:])

            sout = const.tile([HID, 2], f32)
            nc.vector.tensor_copy(out=sout[:, 0:1], in_=acc_w[:])
            nc.vector.tensor_copy(out=sout[:, 1:2], in_=acc_c[:])
            nc.sync.dma_start(out=o_s[:], in_=sout[:])

    _split_multiwaits(nc)
    return nc


def kernel(**inputs):
    xp = np.ascontiguousarray(inputs["x_paper"], np.float32)
    xa = np.ascontiguousarray(inputs["x_author"], np.float32)
    Wp, bp = np.asarray(inputs["W_paper"], np.float32), np.asarray(inputs["b_paper"], np.float32)
    Wa, ba = np.asarray(inputs["W_author"], np.float32), np.asarray(inputs["b_author"], np.float32)
    kW, kb = np.asarray(inputs["k_W"], np.float32), np.asarray(inputs["k_b"], np.float32)
    q = np.asarray(inputs["q"], np.float32)
    lW, lb = np.asarray(inputs["lin_W"], np.float32), np.asarray(inputs["lin_b"], np.float32)
    assert not bp.any() and not ba.any(), "nonzero node bias not supported"

    # combined projection weights
    wcp = np.concatenate([
        Wp,
        _fold_att(Wp, np.asarray(inputs["att_src_cites"], np.float32)),
        _fold_att(Wp, np.asarray(inputs["att_dst_cites"], np.float32)),
        _fold_att(Wp, np.asarray(inputs["att_dst_writes"], np.float32)),
    ], axis=1)
    wca = np.concatenate([
        Wa,
        _fold_att(Wa, np.asarray(inputs["att_src_writes"], np.float32)),
    ], axis=1)

    ew, nbw_w = _prep_edges(np.asarray(inputs["writes_src"]), np.asarray(inputs["writes_dst"]))
    ec, nbw_c = _prep_edges(np.asarray(inputs["cites_src"]), np.asarray(inputs["cites_dst"]))

    nc = _build_program(nbw_w, nbw_c)

    iotam = np.tile(np.arange(P, dtype=np.float32), (P, 1))
    common = {
        "xp": np.ascontiguousarray(xp.T.astype(ml_dtypes.bfloat16)), "xa": np.ascontiguousarray(xa.T.astype(ml_dtypes.bfloat16)),
        "wcp": wcp.astype(ml_dtypes.bfloat16), "wca": wca.astype(ml_dtypes.bfloat16),
        "kw": kW, "kb": kb.reshape(HID, 1).copy(), "lw": lW, "iotam": iotam,
    }
    in_maps = []
    for c in range(N_CORES):
        m = dict(common)
        m.update({
            "src_w": ew[c]["src"], "slot_w": ew[c]["slot"], "ad_w": ew[c]["ad"],
            "src_c": ec[c]["src"], "slot_c": ec[c]["slot"], "ad_c": ec[c]["ad"],
        })
        in_maps.append(m)

    res = run_bass_kernel_spmd(nc, in_maps, list(range(N_CORES)))
    global LAST_RES, LAST_A, LAST_SCORES
    LAST_RES = res

    # host combine: semantic attention softmax + weighted sum + lin_b
    n_pad = NW * P - DST_PER_CORE
    S = np.zeros((HID, 2), np.float64)
    for c in range(N_CORES):
        S += res.results[c]["sacc"].astype(np.float64)
    S -= N_CORES * n_pad * np.tanh(kb)[:, None]  # remove padded-column tanh(k_b)
    t_mean = S / N_PAPER  # [HID, 2]
    scores = q @ t_mean  # [2]
    e = np.exp(scores - scores.max())
    a = e / e.sum()  # [a_writes, a_cites]
    LAST_A, LAST_SCORES = a, scores

    out = np.empty((N_PAPER, EMB), np.float32)
    for c in range(N_CORES):
        yw = res.results[c]["yw"][:, :DST_PER_CORE]
        yc = res.results[c]["yc"][:, :DST_PER_CORE]
        out[c * DST_PER_CORE:(c + 1) * DST_PER_CORE] = (a[0] * yw + a[1] * yc).T
    out += lb[None, :]
    return out
